# revision 38
# baseline (speedup 1.0000x reference)
"""Causal multi-head self-attention block (B=8, T=1024, C=768, H=12) on 8 TRN2
NeuronCores, data-parallel over the batch dimension: core b computes batch b
end-to-end (no collectives).

v2 layout strategy (per core):
  - host passes x[b] pre-transposed: xT [C=768, T=1024]; the 1/sqrt(hd) score
    scale is folded into the q-columns of w_attn on the host, so the qk
    psum->sbuf evacuation is a plain cast when b_attn is zero.
  - qT,kT are produced channel-major ([ch, tok]) with w_attn slices stationary;
    v is produced token-major with xT slices stationary.  attention scores are
    computed transposed, attT[k, q], softmax'd as exp(score) without max
    subtraction (logits ~N(0,0.3)), causal-masked via a triangular tile
    multiply on GPSIMD.
  - softmax denominators ride along as a ones column appended to the
    stationary v operand (65-wide lhsT): row 64 of each AV psum is the sum.
    They are staged to SBUF, reciprocal'd in one batched DVE
    reciprocal_approx_fast per q-slice, partition-broadcast on GPSIMD (no
    DRAM bounce), and multiplied into yT in 12 full-width DVE ops.
  - emission is software-pipelined at head-pair granularity: scores/exp of
    pair j overlap the qkv matmuls of pair j+1, AV of pair j-1, and (during
    the second q-slice) the output projection of the first q-slice, so the
    PE never drains between phases and the exp stream on ACT runs
    continuously from ~15% into the kernel.
  - output is written bf16 and upcast on the host.
"""

from contextlib import ExitStack

import ml_dtypes
import numpy as np

import concourse.bass as bass
import concourse.tile as tile
from concourse import bacc, mybir

N_CORES = 8
B, T, C = 8, 1024, 768
H, HD = 12, 64
C3 = 3 * C
DT = mybir.dt.float32
BF = mybir.dt.bfloat16
AF = mybir.ActivationFunctionType
P = 128
KC = C // P            # 6 k-tiles over the embedding dim
NSUB = (2 * C) // P    # 12 channel blocks covering q and k
TB = T // P            # 8 token blocks
QS = T // 512          # 2 query slices of 512

try:
    from concourse.dve_ops import RECIPROCAL_APPROX_FAST  # noqa: F401
    HAVE_APPROX_RECIP = True
except Exception:
    HAVE_APPROX_RECIP = False


def _emit(tc: tile.TileContext, io: dict, with_qkbias: bool = False,
          with_vbias: bool = False, with_pbias: bool = False) -> None:
    nc = tc.nc
    xT_d, wqk_d, wv_d, bqk_d, bv_d, tri_d, wp_d, bp_d, out_d = (
        io["xT"], io["wqk"], io["wv2"], io["bqk"], io["bv"], io["tri"],
        io["wp"], io["bp"], io["out"],
    )

    stack = ExitStack()
    const = stack.enter_context(tc.tile_pool(name="const", bufs=1))
    persist = stack.enter_context(tc.tile_pool(name="persist", bufs=1))
    work = stack.enter_context(tc.tile_pool(name="work", bufs=1))
    psum = stack.enter_context(tc.tile_pool(name="psum", bufs=1, space="PSUM"))
    dram = stack.enter_context(tc.tile_pool(name="dram", bufs=1, space="DRAM"))

    # ---- persistent tiles ------------------------------------------------
    # qkT[s]: channel-major q/k; s 0-5 -> q channels, 6-11 -> k channels
    qkT = [persist.tile([P, T], BF, tag=f"qkT{s}", name=f"qkT{s}") for s in range(NSUB)]
    # v3[tb]: token-major v, one 128-wide group per head (64 ch + ones col +
    # 63 zero pad): the full-128 stationary keeps Fast Weight Load enabled
    # on the AV matmuls (M=65 would serialize LDWEIGHTS at half rate)
    v3 = [persist.tile([P, H, P], BF, tag=f"v{tb}", name=f"v{tb}") for tb in range(TB)]
    # yT[qs][kc]: channel-major attention output (heads 2k at part 0:64,
    # 2k+1 at 64:128), raw until the finish pass normalizes in place
    yT = [[persist.tile([P, 512], BF, tag=f"yT{q}_{k}", name=f"yT{q}_{k}")
           for k in range(KC)] for q in range(QS)]
    wp = [persist.tile([P, C], BF, tag=f"wp{k}", name=f"wp{k}") for k in range(KC)]
    xT = [[persist.tile([P, 512], BF, tag=f"xT{k}_{i}", name=f"xT{k}_{i}")
           for i in range(2)] for k in range(KC)]
    wsubs = [persist.tile([P, KC, P], BF, tag=f"wsub{s}", name=f"wsub{s}")
             for s in range(NSUB)]
    wv = [persist.tile([P, KC, 384], BF, tag=f"wv{c}", name=f"wv{c}") for c in range(2)]
    # per-pair denominator staging: [2, 512] keeps every DVE/DMA operand at
    # base partition 0 (arbitrary partition bases are not allowed on DVE);
    # short-lived, so they rotate through the work pool (dict keyed by pair)
    sumsP = {}
    # tail pairs normalize via a PE ones-matmul broadcast (no DMA latency);
    # their per-half reciprocals land in dedicated base-0 tiles
    FAST_FIN = {(1, 3), (1, 4), (1, 5)}
    rinvH = {(q, j): [persist.tile([1, 512], BF, tag=f"rh{q}_{j}_{h}",
                                   name=f"rh{q}_{j}_{h}") for h in range(2)]
             for (q, j) in FAST_FIN}

    tri = const.tile([P, P], BF, tag="tri")          # tri[i,j] = j >= i
    ones = const.tile([1, T], BF, tag="ones")
    onesf = const.tile([1, P], DT, tag="onesf")      # fp32 ones (PE broadcast)
    bqk = const.tile([P, NSUB], DT, tag="bqk") if with_qkbias else None
    bv = const.tile([1, C], BF, tag="bv") if with_vbias else None
    bp = const.tile([1, C], BF, tag="bp") if with_pbias else None

    # ---- startup DMAs in first-consumer order ----------------------------
    # qk weights split in k-halves so each accumulation chain can start
    # before its full weight tile lands; the qk chains run half-0 first, so
    # half-1 xT loads issue from the scalar queue in parallel
    nc.sync.dma_start(wsubs[0][:, 0:2, :], wqk_d[0, :, 0:2, :])
    nc.sync.dma_start(xT[0][0][:], xT_d[0:P, 0:512])
    nc.sync.dma_start(wsubs[0][:, 2:KC, :], wqk_d[0, :, 2:KC, :])
    for k in range(1, KC):
        nc.sync.dma_start(xT[k][0][:], xT_d[k * P:(k + 1) * P, 0:512])
    nc.scalar.dma_start(tri[:], tri_d[:, :])
    for k in range(KC):
        nc.scalar.dma_start(xT[k][1][:], xT_d[k * P:(k + 1) * P, 512:1024])
    for s in [KC] + [s for j in range(1, KC) for s in (j, KC + j)]:
        nc.sync.dma_start(wsubs[s][:, 0:3, :], wqk_d[s, :, 0:3, :])
        nc.sync.dma_start(wsubs[s][:, 3:KC, :], wqk_d[s, :, 3:KC, :])
    for c in range(2):  # v weights, split for queue parallelism
        nc.scalar.dma_start(wv[c][:, 0:3, :], wv_d[c, :, 0:3, :])
        nc.scalar.dma_start(wv[c][:, 3:KC, :], wv_d[c, :, 3:KC, :])
    if with_qkbias:
        nc.scalar.dma_start(bqk[:], bqk_d[:, :])
    if with_vbias:
        nc.scalar.dma_start(bv[:], bv_d[:, :])
    if with_pbias:
        nc.scalar.dma_start(bp[:], bp_d[:, :])
    for k in range(KC):  # proj weights (needed ~60% in)
        nc.scalar.dma_start(wp[k][:], wp_d[k * P:(k + 1) * P, :])

    nc.vector.memset(ones[:], 1.0)
    nc.vector.memset(onesf[:], 1.0)
    for tb in range(TB):  # zero pad + ones columns (denominator accumulators)
        nc.vector.memset(v3[tb][:, :, HD:], 0.0)
        nc.vector.memset(v3[tb][:, :, HD:HD + 1], 1.0)

    # warm the PE's HAM clock gate during the startup DMA window: ~3.4us of
    # continuous (tiny) matmul activity lifts the array to 2.4 GHz before
    # the first real chain arrives
    wps = psum.tile([P, 512], DT, tag="pA", name="warm", bufs=2)
    for _ in range(60):
        nc.tensor.matmul(wps[0:HD, 0:HD], lhsT=ones[0:1, 0:HD],
                         rhs=ones[0:1, 0:HD], start=True, stop=True)

    # ---- emit helpers ----------------------------------------------------
    def emit_qk(j):
        """qT/kT channel blocks for head pair j: s=j (q) and s=KC+j (k).

        Half-0 chains run before half-1 so the first chain only needs the
        half-0 xT loads; evacuation stays off ACT (exp owns that engine)."""
        for s in (j, KC + j):
            for i in range(2):
                ps = psum.tile([P, 512], DT, tag="pA", name=f"pA{s}_{i}", bufs=2)
                for k in range(KC):
                    nc.tensor.matmul(
                        ps[:], lhsT=wsubs[s][:, k, :], rhs=xT[k][i][:],
                        start=(k == 0), stop=(k == KC - 1),
                    )
                dst = qkT[s][:, i * 512:(i + 1) * 512]
                if with_qkbias:
                    nc.vector.tensor_scalar(
                        dst, ps[:], 1.0, bqk[:, s:s + 1],
                        mybir.AluOpType.mult, mybir.AluOpType.add,
                    )
                else:
                    nc.vector.tensor_copy(dst, ps[:])

    def emit_v(tbs):
        """token-major v for the given token blocks, xT slice stationary."""
        for tb in tbs:
            for c in range(2):
                pv = psum.tile([P, 512], DT, tag="pA", name="pAv", bufs=2)
                for k in range(KC):
                    nc.tensor.matmul(
                        pv[:, :384],
                        lhsT=xT[k][tb // 4][:, (tb % 4) * P:(tb % 4 + 1) * P],
                        rhs=wv[c][:, k, :], start=(k == 0),
                        stop=(not with_vbias and k == KC - 1),
                    )
                if with_vbias:
                    nc.tensor.matmul(
                        pv[:, :384], lhsT=ones[:, tb * P:(tb + 1) * P],
                        rhs=bv[:, c * 384:(c + 1) * 384], start=False, stop=True,
                    )
                # ACT is idle during the qkv window (exp starts later)
                nc.scalar.activation(
                    v3[tb][:, c * 6:(c + 1) * 6, 0:HD],
                    pv[:, :384].rearrange("p (h f) -> p h f", h=6), AF.Copy,
                )

    def emit_scores(j, qs):
        """score+exp+mask for heads (2j, 2j+1) over all causal k-blocks.

        The two heads live at partition bases 0 and 64 of the same qkT tile,
        so their K=64 matmuls land in different PE row groups and run
        concurrently (tile_position auto-derived from base_partition)."""
        nkb = 4 * (qs + 1)
        ets = []
        for kb in range(nkb):
            # columns < d are causally dead: matmul/exp/av all skip them
            d = max(kb * P - qs * 512, 0)
            pt = psum.tile([P, 1024], DT, tag="sps", name="sps", bufs=2)
            for half in range(2):
                po = half * HD
                nc.tensor.matmul(
                    pt[:, half * 512 + d:(half + 1) * 512],
                    lhsT=qkT[KC + j][po:po + HD, kb * P:(kb + 1) * P],
                    rhs=qkT[j][po:po + HD, qs * 512 + d:(qs + 1) * 512],
                    start=True, stop=True,
                )
            # one ACT op exps both heads' valid columns
            e = work.tile([P, 1024], BF, tag=f"E{kb}", name=f"E{kb}", bufs=2)
            if d == 0:
                nc.scalar.activation(e[:], pt[:], AF.Exp)
            else:
                er = e[:].rearrange("p (a f) -> p a f", a=2)[:, :, d:512]
                pr = pt[:].rearrange("p (a f) -> p a f", a=2)[:, :, d:512]
                nc.scalar.activation(er, pr, AF.Exp)
            if kb * P - qs * 512 >= 0:  # diagonal block: triangular mask
                ed = e[:].rearrange("p (a f) -> p a f", a=2)[:, :, d:d + P]
                nc.gpsimd.tensor_tensor(
                    ed, ed, tri[:, None, :].to_broadcast((P, 2, P)),
                    mybir.AluOpType.mult,
                )
            ets.append(e)
        return ets

    def emit_av(j, qs, ets):
        """unnormalized yT for heads (2j, 2j+1); the fused ones column puts
        each head's softmax denominator in psum row 64, staged into sums[qs]
        via a DVE copy + gpsimd SBUF->SBUF dma (partition relocation)."""
        nkb = len(ets)
        if (qs, j) not in FAST_FIN:
            sumsP[(qs, j)] = work.tile([2, 512], DT, tag="sums", name="sums",
                                       bufs=3)
        for half in range(2):
            h = 2 * j + half
            po, eo = half * HD, half * 512
            psy = psum.tile([P, 512], DT, tag="yps", name="yps", bufs=2)
            for kb in range(nkb):
                d = max(kb * P - qs * 512, 0)
                nc.tensor.matmul(
                    psy[:, d:512], lhsT=v3[kb][:, h, :],
                    rhs=ets[kb][:, eo + d:eo + 512],
                    start=(kb == 0), stop=(kb == nkb - 1),
                )
            nc.vector.tensor_copy(yT[qs][j][po:po + HD, :], psy[0:HD, :])
            stg = work.tile([1, 512], DT, tag="stg", name="stg", bufs=4)
            nc.vector.tensor_copy(stg[:], psy[HD:HD + 1, :])
            if (qs, j) in FAST_FIN:
                # latency-critical tail pairs: reciprocal directly from the
                # staged row; broadcast happens on the PE (emit_finish_pe)
                rt = work.tile([1, 512], DT, tag="rt", name="rt", bufs=2)
                if HAVE_APPROX_RECIP:
                    nc.vector.reciprocal_approx_fast(rt[:], stg[:])
                else:
                    nc.vector.reciprocal(rt[:], stg[:])
                nc.vector.tensor_copy(rinvH[(qs, j)][half][:], rt[:])
            else:
                nc.sync.dma_start(sumsP[(qs, j)][half:half + 1, :], stg[:])

    def emit_finish(qs, j):
        """per-pair reciprocal of the two denominators, partition-broadcast
        via a DRAM bounce (SBUF APs need nonzero partition step), then one
        full-width normalize multiply for channel block j — pipelined right
        behind emit_av(j, qs) so the last pair's normalize is the only one
        on the critical path into the projection."""
        rinvW = work.tile([2, 512], DT, tag="rinvW", name="rinvW", bufs=3)
        if HAVE_APPROX_RECIP:
            nc.vector.reciprocal_approx_fast(rinvW[:], sumsP.pop((qs, j))[:])
        else:
            nc.vector.reciprocal(rinvW[:], sumsP.pop((qs, j))[:])
        rd = dram.tile([2, 512], DT, tag="rd", name="rd", bufs=4)
        nc.gpsimd.dma_start(rd[:], rinvW[:])
        rb = work.tile([P, 512], DT, tag="rb", name="rb", bufs=3)
        for half in range(2):
            nc.sync.dma_start(
                rb[half * HD:(half + 1) * HD, :],
                rd[half:half + 1, :].to_broadcast((HD, 512)),
            )
        nc.vector.tensor_mul(yT[qs][j][:], yT[qs][j][:], rb[:])

    def emit_finish_pe(qs, j):
        """normalize via a K=1 ones-matmul partition broadcast into PSUM —
        zero DMA on the critical path into the final projection."""
        rbps = psum.tile([P, 512], DT, tag="pA", name="pAb", bufs=2)
        for half in range(2):
            po = half * HD
            nc.tensor.matmul(
                rbps[po:po + HD, :], lhsT=ones[0:1, po:po + HD],
                rhs=rinvH[(qs, j)][half][:], start=True, stop=True,
            )
        nc.vector.tensor_mul(yT[qs][j][:], yT[qs][j][:], rbps[:])

    def emit_proj(qs, tbs):
        for tb in tbs:
            for c in range(2):
                pso = psum.tile([P, 512], DT, tag="pA", name="pAo", bufs=2)
                for k in range(KC):
                    nc.tensor.matmul(
                        pso[:, :384],
                        lhsT=yT[qs][k][:, (tb % 4) * P:(tb % 4 + 1) * P],
                        rhs=wp[k][:, c * 384:(c + 1) * 384],
                        start=(k == 0),
                        stop=(not with_pbias and k == KC - 1),
                    )
                if with_pbias:
                    nc.tensor.matmul(
                        pso[:, :384], lhsT=ones[:, tb * P:(tb + 1) * P],
                        rhs=bp[:, c * 384:(c + 1) * 384], start=False, stop=True,
                    )
                osb = work.tile([P, 384], BF, tag="osb", name="osb", bufs=3)
                if qs == 1:  # exp stream is over; ACT is free at the tail
                    nc.scalar.activation(osb[:], pso[:, :384], AF.Copy)
                else:
                    nc.vector.tensor_copy(osb[:], pso[:, :384])
                nc.scalar.dma_start(
                    out_d[tb * P:(tb + 1) * P, c * 384:(c + 1) * 384], osb[:]
                )

    # ---- software-pipelined emission ------------------------------------
    # PE FIFO order interleaves qkv-projection matmuls (no E dependency)
    # between score and AV chains so the PE keeps running while ACT works
    # through the exp stream; proj(qs=0) is threaded into the qs=1 wave.
    E = {}
    emit_qk(0)
    emit_qk(1)
    E[(0, 0)] = emit_scores(0, 0)
    emit_v([0, 1])
    emit_qk(2)
    E[(1, 0)] = emit_scores(1, 0)
    emit_v([2, 3])
    emit_qk(3)
    E[(2, 0)] = emit_scores(2, 0)
    emit_v([4, 5, 6, 7])
    emit_av(0, 0, E.pop((0, 0)))
    emit_finish(0, 0)
    emit_qk(4)
    E[(3, 0)] = emit_scores(3, 0)
    emit_av(1, 0, E.pop((1, 0)))
    emit_finish(0, 1)
    emit_qk(5)
    E[(4, 0)] = emit_scores(4, 0)
    emit_av(2, 0, E.pop((2, 0)))
    emit_finish(0, 2)
    E[(5, 0)] = emit_scores(5, 0)
    emit_av(3, 0, E.pop((3, 0)))
    emit_finish(0, 3)
    emit_av(4, 0, E.pop((4, 0)))
    emit_finish(0, 4)
    emit_av(5, 0, E.pop((5, 0)))
    emit_finish(0, 5)
    E[(0, 1)] = emit_scores(0, 1)
    emit_proj(0, [0])
    E[(1, 1)] = emit_scores(1, 1)
    emit_av(0, 1, E.pop((0, 1)))
    emit_finish(1, 0)
    emit_proj(0, [1])
    E[(2, 1)] = emit_scores(2, 1)
    emit_av(1, 1, E.pop((1, 1)))
    emit_finish(1, 1)
    emit_proj(0, [2])
    E[(3, 1)] = emit_scores(3, 1)
    emit_av(2, 1, E.pop((2, 1)))
    emit_finish(1, 2)
    emit_proj(0, [3])
    E[(4, 1)] = emit_scores(4, 1)
    emit_av(3, 1, E.pop((3, 1)))
    E[(5, 1)] = emit_scores(5, 1)
    emit_av(4, 1, E.pop((4, 1)))
    emit_finish_pe(1, 3)
    emit_av(5, 1, E.pop((5, 1)))
    emit_finish_pe(1, 4)
    emit_finish_pe(1, 5)
    emit_proj(1, [4, 5, 6, 7])

    stack.close()


def build_program(with_qkbias: bool = False, with_vbias: bool = False,
                  with_pbias: bool = False) -> tuple[bass.Bass, dict]:
    nc = bacc.Bacc("TRN2", debug=False)
    io = {
        "xT": nc.dram_tensor("xT", [C, T], BF, kind="ExternalInput"),
        "wqk": nc.dram_tensor("wqk", [NSUB, P, KC, P], BF, kind="ExternalInput"),
        "wv2": nc.dram_tensor("wv2", [2, P, KC, 384], BF, kind="ExternalInput"),
        "bqk": nc.dram_tensor("bqk", [P, NSUB], DT, kind="ExternalInput"),
        "bv": nc.dram_tensor("bv", [1, C], BF, kind="ExternalInput"),
        "tri": nc.dram_tensor("tri", [P, P], BF, kind="ExternalInput"),
        "wp": nc.dram_tensor("wp", [C, C], BF, kind="ExternalInput"),
        "bp": nc.dram_tensor("bp", [1, C], BF, kind="ExternalInput"),
        "out": nc.dram_tensor("out", [T, C], BF, kind="ExternalOutput"),
    }
    with tile.TileContext(nc) as tc:
        _emit(tc, io, with_qkbias=with_qkbias, with_vbias=with_vbias,
              with_pbias=with_pbias)
    nc.compile()
    return nc, io


_CACHED = {}


def make_in_maps(x, w_attn, b_attn, w_proj, b_proj):
    x = np.asarray(x, np.float32)
    w_attn = np.asarray(w_attn, np.float32)
    b_attn = np.asarray(b_attn, np.float32)
    w_proj = np.asarray(w_proj, np.float32)
    b_proj = np.asarray(b_proj, np.float32)

    bf16 = ml_dtypes.bfloat16
    xT = np.ascontiguousarray(x.transpose(0, 2, 1)).astype(bf16)  # [B, C, T]
    bqk = np.ascontiguousarray(
        np.concatenate([b_attn[:C] * 0.125, b_attn[C:2 * C]])
        .reshape(NSUB, P).T
    )                                                        # [P, NSUB] fp32
    bv = b_attn[2 * C:].reshape(1, C).astype(bf16)
    bp = b_proj.reshape(1, C).astype(bf16)
    tri = np.triu(np.ones((P, P), bf16))                     # tri[i,j] = j>=i
    # wqk[s, p, kc, c] = w_attn[kc*128+p, s*128+c], with the 1/sqrt(hd)
    # score scale folded into the q slices (s < 6); wv2[i, p, kc, c] =
    # w_attn[kc*128+p, 1536+i*384+c] -- contiguous SBUF-layout weight loads
    w4 = w_attn.reshape(KC, P, C3)
    wqk = w4[:, :, :2 * C].reshape(KC, P, NSUB, P).transpose(2, 1, 0, 3).copy()
    wqk[:KC] *= 0.125
    wqk = np.ascontiguousarray(wqk).astype(bf16)
    wv2 = np.ascontiguousarray(
        w4[:, :, 2 * C:].reshape(KC, P, 2, 384).transpose(2, 1, 0, 3)
    ).astype(bf16)
    shared = {
        "wqk": wqk, "wv2": wv2, "bqk": bqk, "bv": bv, "tri": tri,
        "wp": np.ascontiguousarray(w_proj).astype(bf16), "bp": bp,
    }
    return [dict(shared, xT=np.ascontiguousarray(xT[b])) for b in range(B)]


def kernel(x, w_attn, b_attn, w_proj, b_proj, _run_kwargs=None):
    from concourse.bass_utils import run_bass_kernel_spmd

    with_qkbias = bool(np.any(np.asarray(b_attn)[:2 * C]))
    with_vbias = bool(np.any(np.asarray(b_attn)[2 * C:]))
    with_pbias = bool(np.any(np.asarray(b_proj)))
    key = ("nc", with_qkbias, with_vbias, with_pbias)
    if key not in _CACHED:
        _CACHED[key] = build_program(with_qkbias, with_vbias, with_pbias)[0]
    nc = _CACHED[key]
    in_maps = make_in_maps(x, w_attn, b_attn, w_proj, b_proj)
    res = run_bass_kernel_spmd(
        nc, in_maps, core_ids=list(range(N_CORES)), **(_run_kwargs or {})
    )
    out = np.stack([np.asarray(res.results[b]["out"]) for b in range(B)]).astype(np.float32)
    if _run_kwargs:
        _CACHED["last_results"] = res
    return out
